# revision 25
# baseline (speedup 1.0000x reference)
"""Trainium2 Bass kernel for nn_JointPairHead: edge gather + LN + 3x(Linear->BN->ReLU) -> logits.

Sharding: data-parallel over E across 8 cores; x and params replicated.
BN batch stats cross-core via AllReduce of per-shard sum/sumsq.

V3 design:
  - bf16 datapath end to end; z resident in SBUF between layers.
  - gather via dma_gather (1024 idx/call, queue-pair rotation 0-3, nothing
    else on gpsimd during phase 0 so emission overlaps across Q7 pairs).
    int16 idx limit handled by 4-class edge partition with lo/hi bases,
    dummy (zero,zero)-edge padding, and analytic stat fixups.
  - all first/second-moment stats via DVE bn_stats (one pass, no separate
    square streams, no accumulator-read overhead); per-block 6-tuples are
    combined into sums at phase end, vectorized over blocks.
  - BN affine folded: h_{i+1} = a_i*relu(z_i - t_i), diag(a_i) folded into
    the next layer's weights on device; relu split across DVE and ACT.
"""

import numpy as np
import ml_dtypes

N_NODES = 50000
D = 256
E_TOT = 262144
NCORES = 8
EBLK = 512
CALL = 1024                     # idx per dma_gather call (= 2 blocks)
H = 32768                       # int16-safe base split
NL = 3
EPS = 1e-5
NROW_PAD = 50004                # [0]=zero, [1..50000]=x, rest zero
DUMLO = 0                       # dummy idx for lo-base gathers (zero row)
DUMHI = 50001 - H               # dummy idx for hi-base gathers (zero row)


def build_nc(caps, num_devices=NCORES):
    """caps: per-class edge capacities (each a multiple of CALL).

    Class c in {0:LL, 1:LH, 2:HL, 3:HH}; src base is hi for c>=2, dst base
    is hi for c%2==1.
    """
    import concourse.bass as bass
    import concourse.mybir as mybir
    import concourse.tile as tile
    from concourse import bacc
    from concourse.masks import make_identity

    f32 = mybir.dt.float32
    bf16 = mybir.dt.bfloat16
    i16 = mybir.dt.int16
    A = mybir.ActivationFunctionType
    ALU = mybir.AluOpType
    AX = mybir.AxisListType

    cap_tot = sum(caps)
    nblk = cap_tot // EBLK
    ncall = cap_tot // CALL
    n_dummy = cap_tot - E_TOT // num_devices   # per-core dummy count
    groups = [list(range(num_devices))]
    inv_d = 1.0 / D
    inv_e = 1.0 / E_TOT

    call_bases = []
    for c in range(4):
        for _ in range(caps[c] // CALL):
            call_bases.append((c >= 2, c % 2 == 1))

    nc = bacc.Bacc("TRN2", target_bir_lowering=False, debug=False,
                   num_devices=num_devices, num_swdge_queues=4)

    # ---- kernel I/O ----
    x = nc.dram_tensor("x", [NROW_PAD, D], bf16, kind="ExternalInput").ap()
    idxs = nc.dram_tensor("idxs", [128, 2 * cap_tot // 16], i16,
                          kind="ExternalInput").ap()
    wts = [nc.dram_tensor(f"w{i}t", [D, D], bf16, kind="ExternalInput").ap()
           for i in range(NL)]
    wot = nc.dram_tensor("wot", [D, 1], bf16, kind="ExternalInput").ap()
    gam = nc.dram_tensor("gam", [NL, D], f32, kind="ExternalInput").ap()
    bet = nc.dram_tensor("bet", [NL, D], f32, kind="ExternalInput").ap()
    out = nc.dram_tensor("out", [nblk * EBLK], f32, kind="ExternalOutput").ap()

    ccin = [nc.dram_tensor(f"ccin{i}", [128, 4], f32, kind="Internal").ap()
            for i in range(NL)]
    cc_space = "Shared" if num_devices > 4 else "Local"
    ccout = [nc.dram_tensor(f"ccout{i}", [128, 4], f32, kind="Internal",
                            addr_space=cc_space).ap()
             for i in range(NL)]

    with tile.TileContext(nc) as tc:
        with (
            tc.tile_pool(name="const", bufs=1) as cpool,
            tc.tile_pool(name="zst", bufs=1) as zpool,
            tc.tile_pool(name="io", bufs=3) as iop,
            tc.tile_pool(name="work", bufs=2) as wp,
            tc.tile_pool(name="stats", bufs=1) as sp,
            tc.tile_pool(name="small", bufs=3) as smp,
            tc.tile_pool(name="htp", bufs=2) as htp,
            tc.tile_pool(name="psum_t", bufs=1, space="PSUM") as ppt,
            tc.tile_pool(name="psum_z", bufs=3, space="PSUM") as ppz,
            tc.tile_pool(name="psum_s", bufs=1, space="PSUM") as pps,
        ):
            # ---- constants ----
            ident = cpool.tile([128, 128], bf16, name="ident")
            make_identity(nc, ident[:])
            idx_sb = cpool.tile([128, 2 * cap_tot // 16], i16, name="idx_sb")
            nc.sync.dma_start(out=idx_sb[:], in_=idxs[:])
            wsb = []
            for i in range(NL):
                chunks = []
                for c in range(2):
                    t = cpool.tile([128, D], bf16, name=f"w{i}c{c}")
                    nc.sync.dma_start(out=t[:], in_=wts[i][c * 128:(c + 1) * 128, :])
                    chunks.append(t)
                wsb.append(chunks)
            wot_sb = []
            for c in range(2):
                t = cpool.tile([128, 1], bf16, name=f"wo{c}")
                nc.sync.dma_start(out=t[:], in_=wot[c * 128:(c + 1) * 128, :])
                wot_sb.append(t)
            wscl = [[cpool.tile([128, D], bf16, name=f"ws{i}c{c}")
                     for c in range(2)] for i in range(1, NL)]
            wos = [cpool.tile([128, 1], bf16, name=f"wos{c}") for c in range(2)]
            gam_sb, bet_sb = [], []
            for i in range(NL):
                g = cpool.tile([128, 2], f32, name=f"gam{i}")
                b = cpool.tile([128, 2], f32, name=f"bet{i}")
                for c in range(2):
                    nc.sync.dma_start(out=g[:, c:c + 1],
                                      in_=gam[i, c * 128:(c + 1) * 128])
                    nc.sync.dma_start(out=b[:, c:c + 1],
                                      in_=bet[i, c * 128:(c + 1) * 128])
                gam_sb.append(g)
                bet_sb.append(b)

            # ---- persistent state ----
            zst = [zpool.tile([128, 2 * EBLK], bf16, name=f"z_{b}")
                   for b in range(nblk)]
            # z bn-stats sheets: per (layer, chunk): [128, 6*nblk]
            zbn = [sp.tile([128, 2, 6 * nblk], f32, name=f"zbn{i}")
                   for i in range(NL)]
            t_ab = [sp.tile([128, 2], f32, name=f"t{i}") for i in range(NL)]
            tneg = [sp.tile([128, 2], f32, name=f"tn{i}") for i in range(NL)]
            a_ab = [sp.tile([128, 2], f32, name=f"a{i}") for i in range(NL)]
            mdum = [sp.tile([128, 2], f32, name=f"md{i}") for i in range(NL)]

            # ================= Phase 0: gather + LN + layer 0 =================
            for ci in range(ncall):
                src_hi, dst_hi = call_bases[ci]
                xs = iop.tile([128, CALL // 128, D], bf16, name="xs", tag="xs")
                xd = iop.tile([128, CALL // 128, D], bf16, name="xd", tag="xd")
                co = CALL // 16
                nc.gpsimd.dma_gather(
                    xs[:], x[H:, :] if src_hi else x[:, :],
                    idx_sb[:, (2 * ci) * co:(2 * ci + 1) * co],
                    CALL, CALL, D, queue_num=(2 * ci) % 4)
                nc.gpsimd.dma_gather(
                    xd[:], x[H:, :] if dst_hi else x[:, :],
                    idx_sb[:, (2 * ci + 1) * co:(2 * ci + 2) * co],
                    CALL, CALL, D, queue_num=(2 * ci + 1) % 4)

                # [128, 4*CB groups, 6] ln stats for the CB blocks of this call
                CB = CALL // EBLK
                lnbn = smp.tile([128, 4 * CB, 6], f32, name="lnbn", tag="lnbn")
                hs = []
                for b2 in range(CB):
                    h = wp.tile([128, 4, D], bf16, name="h", tag=f"h{b2}")
                    nc.vector.tensor_add(out=h[:],
                                         in0=xs[:, 4 * b2:4 * b2 + 4, :],
                                         in1=xd[:, 4 * b2:4 * b2 + 4, :])
                    # walrus requires bn_stats out == 6 elems/partition
                    for g in range(4):
                        nc.vector.bn_stats(
                            out=lnbn[:, 4 * b2 + g, :],
                            in_=h[:, g, :])
                    hs.append(h)
                # LN math on [128, 8] strided views.
                # mu = (me+mo)/2 ;  var = (ve+vo)/D + (me^2+mo^2)/2 - mu^2
                # with me^2+mo^2 = s^2 - 2*me*mo, s = me+mo, mu^2 = s^2/4.
                me = lnbn[:, :, 1]
                ve = lnbn[:, :, 2]
                mo = lnbn[:, :, 4]
                vo = lnbn[:, :, 5]
                s = smp.tile([128, 4 * CB], f32, name="s", tag="s")
                p = smp.tile([128, 4 * CB], f32, name="p", tag="p")
                s2 = smp.tile([128, 4 * CB], f32, name="s2", tag="s2")
                var = smp.tile([128, 4 * CB], f32, name="var", tag="var")
                mu = smp.tile([128, 4 * CB], f32, name="mu", tag="mu")
                inv = smp.tile([128, 4 * CB], f32, name="inv", tag="inv")
                rs = smp.tile([128, 4 * CB], f32, name="rs", tag="rs")
                bneg = smp.tile([128, 4 * CB], f32, name="bneg", tag="bneg")
                nc.vector.scalar_tensor_tensor(
                    out=s[:], in0=me, scalar=1.0, in1=mo,
                    op0=ALU.mult, op1=ALU.add)
                nc.vector.scalar_tensor_tensor(
                    out=p[:], in0=me, scalar=1.0, in1=mo,
                    op0=ALU.mult, op1=ALU.mult)
                nc.vector.scalar_tensor_tensor(
                    out=s2[:], in0=s[:], scalar=1.0, in1=s[:],
                    op0=ALU.mult, op1=ALU.mult)
                # var0 = (ve+vo)/D
                nc.vector.scalar_tensor_tensor(
                    out=var[:], in0=ve, scalar=1.0, in1=vo,
                    op0=ALU.mult, op1=ALU.add)
                nc.vector.tensor_scalar(out=var[:], in0=var[:], scalar1=inv_d,
                                        scalar2=None, op0=ALU.mult)
                # var += 0.5*(s^2 - 2p) - 0.25*s^2 = 0.25*s^2 - p
                nc.vector.scalar_tensor_tensor(
                    out=var[:], in0=s2[:], scalar=0.25, in1=var[:],
                    op0=ALU.mult, op1=ALU.add)
                nc.vector.tensor_sub(out=var[:], in0=var[:], in1=p[:])
                nc.vector.tensor_scalar(out=var[:], in0=var[:], scalar1=EPS,
                                        scalar2=None, op0=ALU.add)
                nc.vector.reciprocal(out=inv[:], in_=var[:])
                nc.scalar.sqrt(out=rs[:], in_=inv[:])
                nc.vector.tensor_scalar(out=mu[:], in0=s[:], scalar1=0.5,
                                        scalar2=None, op0=ALU.mult)
                nc.vector.scalar_tensor_tensor(
                    out=bneg[:], in0=mu[:], scalar=-1.0, in1=rs[:],
                    op0=ALU.mult, op1=ALU.mult)

                for b2 in range(CB):
                    blk = CB * ci + b2
                    h = hs[b2]
                    # LN apply in place: split groups DVE/ACT
                    for g in range(4):
                        gg = b2 * 4 + g
                        if g < 2:
                            nc.vector.tensor_scalar(
                                out=h[:, g, :], in0=h[:, g, :],
                                scalar1=rs[:, gg:gg + 1],
                                scalar2=bneg[:, gg:gg + 1],
                                op0=ALU.mult, op1=ALU.add)
                        else:
                            nc.scalar.activation(
                                out=h[:, g, :], in_=h[:, g, :], func=A.Identity,
                                bias=bneg[:, gg:gg + 1], scale=rs[:, gg:gg + 1])
                    hT = []
                    for c in range(2):
                        tp = ppt.tile([128, EBLK], bf16, name="tp", tag="tp")
                        for g in range(4):
                            nc.tensor.transpose(
                                out=tp[:, g * 128:(g + 1) * 128],
                                in_=h[:, g, c * 128:(c + 1) * 128],
                                identity=ident[:])
                        hc = htp.tile([128, EBLK], bf16, name=f"hTc{c}",
                                      tag=f"hT{c}")
                        if c == 0:
                            nc.scalar.copy(out=hc[:], in_=tp[:])
                        else:
                            nc.vector.tensor_copy(hc[:], tp[:])
                        hT.append(hc)
                    zps = ppz.tile([128, 2 * EBLK], f32, name="zps", tag="zps")
                    for j in range(2):
                        for c in range(2):
                            nc.tensor.matmul(
                                out=zps[:, j * EBLK:(j + 1) * EBLK],
                                lhsT=wsb[0][c][:, j * 128:(j + 1) * 128],
                                rhs=hT[c][:], start=(c == 0), stop=(c == 1))
                    nc.scalar.copy(out=zst[blk][:], in_=zps[:])
                    for j in range(2):
                        nc.vector.bn_stats(
                            out=zbn[0][:, j, 6 * blk:6 * blk + 6],
                            in_=zst[blk][:, j * EBLK:(j + 1) * EBLK])

            # ====== stats AllReduce + BN affine + weight fold ======
            scr_sh = sp.tile([128, 2, nblk], f32, name="scr_sh")
            scr2_sh = sp.tile([128, 2, nblk], f32, name="scr2_sh")

            def finalize_stats(li):
                # combine per-block bn 6-tuples into sum / sumsq, per chunk
                st4 = sp.tile([128, 4], f32, name=f"st4_{li}")
                v = zbn[li][:].rearrange("p c (b s) -> p c b s", s=6)
                me, ve = v[:, :, :, 1], v[:, :, :, 2]
                mo, vo = v[:, :, :, 4], v[:, :, :, 5]
                rs_ = sp.tile([128, 2], f32, name=f"rs_{li}")
                rq = sp.tile([128, 2], f32, name=f"rq{li}")
                rs2 = sp.tile([128, 2], f32, name=f"rs2_{li}")
                rp = sp.tile([128, 2], f32, name=f"rp{li}")
                msq_ = sp.tile([128, 2], f32, name=f"msq_{li}")
                # s = me+mo ; q = ve+vo ; s2 = s*s ; p = me*mo
                nc.vector.scalar_tensor_tensor(
                    out=scr_sh[:], in0=me, scalar=1.0, in1=mo,
                    op0=ALU.mult, op1=ALU.add)
                nc.vector.reduce_sum(out=rs_[:], in_=scr_sh[:], axis=AX.X)
                nc.vector.scalar_tensor_tensor(
                    out=scr2_sh[:], in0=ve, scalar=1.0, in1=vo,
                    op0=ALU.mult, op1=ALU.add)
                nc.vector.reduce_sum(out=rq[:], in_=scr2_sh[:], axis=AX.X)
                nc.vector.scalar_tensor_tensor(
                    out=scr2_sh[:], in0=scr_sh[:], scalar=1.0, in1=scr_sh[:],
                    op0=ALU.mult, op1=ALU.mult)
                nc.vector.reduce_sum(out=rs2[:], in_=scr2_sh[:], axis=AX.X)
                nc.vector.scalar_tensor_tensor(
                    out=scr_sh[:], in0=me, scalar=1.0, in1=mo,
                    op0=ALU.mult, op1=ALU.mult)
                nc.vector.reduce_sum(out=rp[:], in_=scr_sh[:], axis=AX.X)
                # sum_z = 256*S(s) ; sumsq = S(q) + 256*(S(s2) - 2*S(p))
                nc.vector.tensor_scalar(
                    out=st4[:, 0:2], in0=rs_[:], scalar1=256.0,
                    scalar2=None, op0=ALU.mult)
                nc.vector.scalar_tensor_tensor(
                    out=msq_[:], in0=rp[:], scalar=-2.0, in1=rs2[:],
                    op0=ALU.mult, op1=ALU.add)
                nc.vector.scalar_tensor_tensor(
                    out=st4[:, 2:4], in0=msq_[:], scalar=256.0, in1=rq[:],
                    op0=ALU.mult, op1=ALU.add)
                if li >= 1:
                    m = mdum[li]
                    msq = sp.tile([128, 2], f32, name=f"msq{li}")
                    nc.vector.scalar_tensor_tensor(
                        out=msq[:], in0=m[:], scalar=1.0, in1=m[:],
                        op0=ALU.mult, op1=ALU.mult)
                    nc.vector.scalar_tensor_tensor(
                        out=st4[:, 0:2], in0=m[:], scalar=-float(n_dummy),
                        in1=st4[:, 0:2], op0=ALU.mult, op1=ALU.add)
                    nc.vector.scalar_tensor_tensor(
                        out=st4[:, 2:4], in0=msq[:], scalar=-float(n_dummy),
                        in1=st4[:, 2:4], op0=ALU.mult, op1=ALU.add)
                nc.sync.dma_start(out=ccin[li][:, :], in_=st4[:])
                if num_devices == 1:
                    nc.sync.dma_start(out=ccout[li][:, :], in_=ccin[li][:, :])
                else:
                    nc.gpsimd.collective_compute(
                        "AllReduce", mybir.AluOpType.add, replica_groups=groups,
                        ins=[ccin[li][:, :]], outs=[ccout[li][:, :]])
                gst = sp.tile([128, 4], f32, name=f"gst{li}")
                nc.sync.dma_start(out=gst[:], in_=ccout[li][:, :])
                bmu = sp.tile([128, 2], f32, name=f"bmu{li}")
                bmu2 = sp.tile([128, 2], f32, name=f"bmu2{li}")
                bvar = sp.tile([128, 2], f32, name=f"bvar{li}")
                binv = sp.tile([128, 2], f32, name=f"binv{li}")
                brs = sp.tile([128, 2], f32, name=f"brs{li}")
                boa = sp.tile([128, 2], f32, name=f"boa{li}")
                nc.scalar.mul(out=bmu[:], in_=gst[:, 0:2], mul=inv_e)
                nc.scalar.square(out=bmu2[:], in_=bmu[:])
                nc.vector.scalar_tensor_tensor(
                    out=bvar[:], in0=gst[:, 2:4], scalar=inv_e, in1=bmu2[:],
                    op0=ALU.mult, op1=ALU.subtract)
                nc.vector.tensor_scalar(out=bvar[:], in0=bvar[:], scalar1=EPS,
                                        scalar2=None, op0=ALU.add)
                nc.vector.reciprocal(out=binv[:], in_=bvar[:])
                nc.scalar.sqrt(out=brs[:], in_=binv[:])
                nc.vector.tensor_mul(out=a_ab[li][:], in0=gam_sb[li][:],
                                     in1=brs[:])
                ainv = sp.tile([128, 2], f32, name=f"ainv{li}")
                nc.vector.reciprocal(out=ainv[:], in_=a_ab[li][:])
                nc.vector.tensor_mul(out=boa[:], in0=bet_sb[li][:], in1=ainv[:])
                nc.vector.tensor_sub(out=t_ab[li][:], in0=bmu[:], in1=boa[:])
                nc.vector.tensor_scalar(out=tneg[li][:], in0=t_ab[li][:],
                                        scalar1=-1.0, scalar2=None,
                                        op0=ALU.mult)
                if li < NL - 1:
                    for c in range(2):
                        nc.vector.tensor_scalar(
                            out=wscl[li][c][:], in0=wsb[li + 1][c][:],
                            scalar1=a_ab[li][:, c:c + 1], scalar2=None,
                            op0=ALU.mult)
                else:
                    for c in range(2):
                        nc.vector.tensor_scalar(
                            out=wos[c][:], in0=wot_sb[c][:],
                            scalar1=a_ab[li][:, c:c + 1], scalar2=None,
                            op0=ALU.mult)
                if li < NL - 1:
                    zd = mdum[li] if li >= 1 else None
                    hd = sp.tile([128, 2], bf16, name=f"hd{li}")
                    if zd is None:
                        nc.vector.tensor_scalar(
                            out=hd[:], in0=t_ab[li][:], scalar1=-1.0,
                            scalar2=0.0, op0=ALU.mult, op1=ALU.max)
                    else:
                        nc.vector.scalar_tensor_tensor(
                            out=hd[:], in0=zd[:], scalar=0.0, in1=t_ab[li][:],
                            op0=ALU.add, op1=ALU.subtract)
                        nc.vector.tensor_scalar(
                            out=hd[:], in0=hd[:], scalar1=0.0, scalar2=None,
                            op0=ALU.max)
                    mps = pps.tile([128, 2], f32, name=f"mps{li}", tag="aux")
                    for j in range(2):
                        for c in range(2):
                            nc.tensor.matmul(
                                out=mps[:, j:j + 1],
                                lhsT=wscl[li][c][:, j * 128:(j + 1) * 128],
                                rhs=hd[:, c:c + 1],
                                start=(c == 0), stop=(c == 1))
                    nc.vector.tensor_copy(mdum[li + 1][:], mps[:])

            finalize_stats(0)

            # ================= layers 1..NL-1 =================
            for li in range(1, NL):
                for bp in range(nblk // 2):
                    hTs = []
                    for b2 in range(2):
                        blk = 2 * bp + b2
                        pair = []
                        for c in range(2):
                            hc = htp.tile([128, EBLK], bf16, name=f"rhc{c}",
                                          tag=f"hT{c}")
                            if c == 0:
                                nc.vector.tensor_scalar(
                                    out=hc[:], in0=zst[blk][:, :EBLK],
                                    scalar1=t_ab[li - 1][:, c:c + 1],
                                    scalar2=0.0,
                                    op0=ALU.subtract, op1=ALU.max)
                            else:
                                nc.scalar.activation(
                                    out=hc[:], in_=zst[blk][:, EBLK:],
                                    func=A.Relu,
                                    bias=tneg[li - 1][:, c:c + 1], scale=1.0)
                            pair.append(hc)
                        hTs.append(pair)
                    zpss = [ppz.tile([128, 2 * EBLK], f32, name="zps",
                                     tag="zps") for _ in range(2)]
                    for j in range(2):
                        for c in range(2):
                            for b2 in range(2):
                                nc.tensor.matmul(
                                    out=zpss[b2][:, j * EBLK:(j + 1) * EBLK],
                                    lhsT=wscl[li - 1][c][:,
                                                        j * 128:(j + 1) * 128],
                                    rhs=hTs[b2][c][:],
                                    start=(c == 0), stop=(c == 1))
                    for b2 in range(2):
                        blk = 2 * bp + b2
                        nc.scalar.copy(out=zst[blk][:], in_=zpss[b2][:])
                        for j in range(2):
                            nc.vector.bn_stats(
                                out=zbn[li][:, j, 6 * blk:6 * blk + 6],
                                in_=zst[blk][:, j * EBLK:(j + 1) * EBLK])
                finalize_stats(li)

            # ================= final projection =================
            OB = 2   # blocks of logits per output DMA
            lsb = None
            for blk in range(nblk):
                lps = pps.tile([1, EBLK], f32, name="lps", tag="aux")
                for c in range(2):
                    hc = htp.tile([128, EBLK], bf16, name=f"fhc{c}",
                                  tag=f"hT{c}")
                    if c == 0:
                        nc.vector.tensor_scalar(
                            out=hc[:], in0=zst[blk][:, :EBLK],
                            scalar1=t_ab[NL - 1][:, c:c + 1], scalar2=0.0,
                            op0=ALU.subtract, op1=ALU.max)
                    else:
                        nc.scalar.activation(
                            out=hc[:], in_=zst[blk][:, EBLK:], func=A.Relu,
                            bias=tneg[NL - 1][:, c:c + 1], scale=1.0)
                    nc.tensor.matmul(out=lps[:], lhsT=wos[c][:], rhs=hc[:],
                                     start=(c == 0), stop=(c == 1))
                ob = blk % OB
                if ob == 0:
                    lsb = wp.tile([1, OB * EBLK], f32, name="lsb", tag="lsb")
                nc.scalar.copy(out=lsb[:, ob * EBLK:(ob + 1) * EBLK], in_=lps[:])
                if ob == OB - 1 or blk == nblk - 1:
                    base = (blk - ob) * EBLK
                    nc.sync.dma_start(
                        out=out[base:base + (ob + 1) * EBLK],
                        in_=lsb[:, :(ob + 1) * EBLK])

    nc.compile()
    return nc


_NC = None
_NC_CAPS = None
_last_in_maps = None


def _wrap16(v):
    return np.ascontiguousarray(v.reshape(-1, 16).T)


def kernel(**inputs):
    global _NC, _NC_CAPS, _last_in_maps
    from concourse import bass_utils

    x = np.asarray(inputs["x"], dtype=np.float32)
    ei = np.asarray(inputs["jg_edge_index"]).astype(np.int64)
    ln_w = np.asarray(inputs["ln_w"], dtype=np.float32)
    Ws = np.asarray(inputs["Ws"], dtype=np.float32)
    gammas = np.asarray(inputs["gammas"], dtype=np.float32)
    betas = np.asarray(inputs["betas"], dtype=np.float32)
    W_out = np.asarray(inputs["W_out"], dtype=np.float32)

    x_pad = np.zeros((NROW_PAD, D), dtype=ml_dtypes.bfloat16)
    x_pad[1:N_NODES + 1] = x.astype(ml_dtypes.bfloat16)

    s1 = ei[0] + 1
    d1 = ei[1] + 1
    cls = (s1 >= H).astype(np.int64) * 2 + (d1 >= H).astype(np.int64)
    by_class = [np.nonzero(cls == c)[0] for c in range(4)]
    percore = [[by_class[c][k::NCORES] for k in range(NCORES)]
               for c in range(4)]
    caps = []
    for c in range(4):
        m = max(len(percore[c][k]) for k in range(NCORES))
        caps.append(int(-(-m // CALL)) * CALL if m > 0 else 0)
    caps = tuple(caps)

    W0f = Ws[0] * ln_w[None, :]
    wts = [np.ascontiguousarray(W0f.T).astype(ml_dtypes.bfloat16),
           np.ascontiguousarray(Ws[1].T).astype(ml_dtypes.bfloat16),
           np.ascontiguousarray(Ws[2].T).astype(ml_dtypes.bfloat16)]
    wot = np.ascontiguousarray(W_out.reshape(1, D).T).astype(ml_dtypes.bfloat16)

    if _NC is None or _NC_CAPS != caps:
        _NC = build_nc(list(caps))
        _NC_CAPS = caps

    in_maps = []
    slots_all = []
    for k in range(NCORES):
        slots = []
        idx_stream = []
        for c in range(4):
            ids = percore[c][k]
            n = len(ids)
            pad = caps[c] - n
            slots.append(np.concatenate([ids, -np.ones(pad, dtype=np.int64)]))
            src_hi, dst_hi = c >= 2, c % 2 == 1
            sv = s1[ids] - (H if src_hi else 0)
            dv = d1[ids] - (H if dst_hi else 0)
            sdum = DUMHI if src_hi else DUMLO
            ddum = DUMHI if dst_hi else DUMLO
            sv = np.concatenate([sv, np.full(pad, sdum, dtype=np.int64)])
            dv = np.concatenate([dv, np.full(pad, ddum, dtype=np.int64)])
            for i in range(caps[c] // CALL):
                idx_stream.append(_wrap16(sv[i * CALL:(i + 1) * CALL]))
                idx_stream.append(_wrap16(dv[i * CALL:(i + 1) * CALL]))
        slots_all.append(np.concatenate(slots))
        idx16 = np.concatenate(idx_stream, axis=1).astype(np.int16)
        idx_t = np.ascontiguousarray(np.tile(idx16, (8, 1)))
        in_maps.append({
            "x": x_pad,
            "idxs": idx_t,
            "w0t": wts[0], "w1t": wts[1], "w2t": wts[2], "wot": wot,
            "gam": gammas, "bet": betas,
        })
    _last_in_maps = in_maps
    res = bass_utils.run_bass_kernel_spmd(_NC, in_maps,
                                          core_ids=list(range(NCORES)))
    logits = np.empty(E_TOT, dtype=np.float32)
    for k in range(NCORES):
        vals = res.results[k]["out"]
        sl = slots_all[k]
        m = sl >= 0
        logits[sl[m]] = vals[m]
    return logits
